# revision 6
# baseline (speedup 1.0000x reference)
"""Trainium2 Bass kernel for nn_DA_affinity_attention (gnn_message_passing).

Math (per batch b):
  coord_aff[n,m,t] = exp(-|q_coord[n,t] - kv_coord[m,t]|)
  for i in 0..1:
    q_  = q  @ Wq[i].T                  # [Nq, 32]
    kv_ = kv @ Wkv[i].T ; k, v = split  # [Nkv, 32] each
    s[n,m] = (sum_c exp(-|q_[n,c]-k[m,c]|) + sum_t wds[i][t]*coord_aff[n,m,t]) / 32
    attn   = softmax(s, axis=m)
    q      = attn @ v
  out = q @ Wp.T + bp

Algorithm: the elementwise affinity reduction over [128 x 2048 x 32] pairs is
replaced by a low-rank Chebyshev factorization

    exp(-|x - y|) ~= sum_{l<L,r<R} B[l,r] T_l(x/sx) T_r(y/sy)

so sum_c aff becomes ONE TensorE matmul with contract dim 32*R (+ coord
channels 3*Rc), i.e. S = G^T.T @ V with
    V[(r,c), m] = T_r(k_c[m]/sy)        (k-side Chebyshev planes)
    G[(r,c), n] = sum_l B[l,r] T_l(q_c[n]/sx)   (q-side, B folded via TensorE)
Iteration 2's q is tiny (|q|<0.05, it is an attention-average), so iter 2 is
linearized instead:  sum_c e^{-|q-k|} ~= sum_c f_c + sum_c q_c g_c  with
f = e^{-|k|}, g = sign(k) e^{-|k|}  -> a rank-64 matmul, no Chebyshev needed.
The coord term uses the same factorization (3 channels, its own fit Bc), with
the per-iteration wds[i] weights folded into the q-side matrices on host.

V planes are built n-major ((r,c) on partitions, m on free dim) with a
stride-4 Chebyshev recurrence: chunk_{j+1} = (2 T_4(x)) o chunk_j - chunk_{j-1}
on full [128, 2048] f16 DVE ops; T_4(x) is replicated across the 4 row groups
by a TensorE matmul with a stacked-identity matrix. Coord V (3 channels) is
built m-major with tiny ops and transposed through TensorE. softmax has no
max-subtraction (scores bounded (0,1]); attn@[v|ones] gives numerator and
denominator in one accumulation; all small projections keep transposed
layouts so no extra transposes are needed anywhere else.

Sharding: B*Nq = 1024 query rows -> 128 rows per core (8 cores), each core
holds the full kv/kv_coord of its batch. Pure SPMD, no collectives.
"""

import sys
from contextlib import ExitStack

for _p in ("/opt/trn_rl_repo",):
    if _p not in sys.path:
        sys.path.insert(0, _p)

import numpy as np
import numpy.polynomial.chebyshev as cheb

import concourse.bacc as bacc
import concourse.bass as bass
import concourse.mybir as mybir
import concourse.tile as tile
from concourse.bass_utils import run_bass_kernel_spmd
from concourse.masks import make_identity

B, NQ, NKV = 2, 512, 2048
C = 32          # ERP_DIM
ICO = 64        # ICO_DIM
ITERS = 2
P = 128         # query rows per core
NCORES = 8
NT = NKV // P   # kv tiles of 128
SCALE = 1.0 / C

# factorization ranks / ranges (fit below is data-independent)
L = 16          # q-side Chebyshev levels
R = 12          # k-side Chebyshev levels (main affinity)
RC = 12         # coord levels
SX1 = 2.7       # iter-1 q_ range scale
SY = 2.9        # k range scale
SXC = 3.6       # q_coord range scale
SYC = 4.3       # kv_coord range scale
NCH = 3 * RC    # coord contract rows (36)

F32 = mybir.dt.float32
F16 = mybir.dt.float16
AF = mybir.ActivationFunctionType
OP = mybir.AluOpType


def _fit_B(Lv, Rv, sx, sy, ngrid=800):
    """LSQ fit of exp(-|x-y|) = sum_{l,r} B[l,r] T_l(x/sx) T_r(y/sy)."""
    x = sx * np.cos(np.pi * (np.arange(ngrid) + 0.5) / ngrid)
    y = sy * np.cos(np.pi * (np.arange(ngrid) + 0.5) / ngrid)
    F = np.exp(-np.abs(x[:, None] - y[None, :]))
    Tx = cheb.chebvander(x / sx, Lv - 1)
    Ty = cheb.chebvander(y / sy, Rv - 1)
    Bm = np.linalg.lstsq(Tx, F, rcond=None)[0]
    Bm = np.linalg.lstsq(Ty, Bm.T, rcond=None)[0].T
    return Bm  # [Lv, Rv]


_B1 = _fit_B(L, R, SX1, SY)
_BC = _fit_B(L, RC, SXC, SYC)

# wpack column layout (f16 canvas [128, WPACK_COLS])
_OFF_WQ1 = 0          # [32 rows, 32] Wq[0].T / SX1
_OFF_WQ2 = 32         # [32, 32] Wq[1].T
_OFF_WK1 = 64         # [64, 32] Wkv[0][:C].T / SY
_OFF_WV1 = 96         # [64, 32] Wkv[0][C:].T
_OFF_WK2 = 128        # [64, 32] Wkv[1][:C].T
_OFF_WV2 = 160        # [64, 32] Wkv[1][C:].T
_OFF_WP = 192         # [32, 32] Wp.T
_OFF_REP = 224        # [32, 128] I32x4*2 replicate matrix
_OFF_MC1 = 352        # [48, 36] coord fold iter-1
_OFF_MC2 = 388        # [48, 36] coord fold iter-2
_OFF_M1 = 424         # [128, 12*128] main fold blocks
WPACK_COLS = _OFF_M1 + (R // 4) * (L // 4) * 128  # 424 + 12*128 = 1960


def build_program(reps=1):
    nc = bacc.Bacc("TRN2", target_bir_lowering=False, debug=False)

    qT_d = nc.dram_tensor("qT16", [C, P], F16, kind="ExternalInput")
    qc_d = nc.dram_tensor("qc_n", [P, 3], F16, kind="ExternalInput")
    kvT_d = nc.dram_tensor("kvT16", [ICO, NKV], F16, kind="ExternalInput")
    kvc_d = nc.dram_tensor("kvc_m", [P, NT * 3], F16, kind="ExternalInput")
    wp_d = nc.dram_tensor("wpack", [P, WPACK_COLS], F16, kind="ExternalInput")
    bpb_d = nc.dram_tensor("bpb", [P, C], F32, kind="ExternalInput")
    y_d = nc.dram_tensor("y", [P, C], F32, kind="ExternalOutput")

    NJ = R // 4   # main V chunks (3)
    NI = L // 4   # q-side feature chunks (4)

    with tile.TileContext(nc) as tc, ExitStack() as ctx:
        sb = ctx.enter_context(tc.tile_pool(name="sb", bufs=1))
        sb2 = ctx.enter_context(tc.tile_pool(name="sb2", bufs=2))
        psBig = ctx.enter_context(tc.tile_pool(name="psBig", bufs=1, space="PSUM"))
        psMix = ctx.enter_context(tc.tile_pool(name="psMix", bufs=2, space="PSUM"))

        if reps > 1:
            _loop = tc.For_i(0, reps, 1)
            _loop.__enter__()

        # ------------------------------------------------ input DMAs
        wpk = sb.tile([P, WPACK_COLS], F16, tag="wpk")
        nc.scalar.dma_start(out=wpk, in_=wp_d.ap())
        kvT = sb.tile([ICO, NKV], F16, tag="kvT")
        nc.sync.dma_start(out=kvT, in_=kvT_d.ap())
        qT = sb.tile([C, P], F16, tag="qT")
        nc.sync.dma_start(out=qT, in_=qT_d.ap())
        qc = sb.tile([P, 3], F16, tag="qc")
        nc.sync.dma_start(out=qc, in_=qc_d.ap())
        kvc = sb.tile([P, NT * 3], F16, tag="kvc")
        nc.sync.dma_start(out=kvc, in_=kvc_d.ap())
        bpb = sb.tile([P, C], F32, tag="bpb")
        nc.sync.dma_start(out=bpb, in_=bpb_d.ap())

        ident = sb.tile([P, P], F16, tag="ident")
        make_identity(nc, ident)

        # weight views
        wq1T = wpk[0:C, _OFF_WQ1:_OFF_WQ1 + C]
        wq2T = wpk[0:C, _OFF_WQ2:_OFF_WQ2 + C]
        wk1T = wpk[0:ICO, _OFF_WK1:_OFF_WK1 + C]
        wv1T = wpk[0:ICO, _OFF_WV1:_OFF_WV1 + C]
        wk2T = wpk[0:ICO, _OFF_WK2:_OFF_WK2 + C]
        wv2T = wpk[0:ICO, _OFF_WV2:_OFF_WV2 + C]
        wpT = wpk[0:C, _OFF_WP:_OFF_WP + C]
        repM = wpk[0:C, _OFF_REP:_OFF_REP + P]
        mc1 = wpk[0:NCH + RC, _OFF_MC1:_OFF_MC1 + NCH]
        mc2 = wpk[0:NCH + RC, _OFF_MC2:_OFF_MC2 + NCH]

        # ------------------------------------------------ coord V  [36, 2048]
        # m-major Wc [128, (t, r, tc)] built by tiny recurrences, then 16
        # per-m-tile transposes into Vc rows (r*3 + tc).
        Wc = sb.tile([P, NT * NCH], F16, tag="Wc")
        Wc3 = Wc[:, :].rearrange("p (t r) -> p t r", t=NT, r=NCH)

        def wc_sl(r):  # [128, (t,3)] slice at level r
            return Wc3[:, :, 3 * r:3 * r + 3]

        kvc3 = kvc[:, :].rearrange("p (t c) -> p t c", t=NT, c=3)
        X2c = sb.tile([P, NT * 3], F16, tag="X2c")
        X2c3 = X2c[:, :].rearrange("p (t c) -> p t c", t=NT, c=3)
        nc.vector.memset(wc_sl(0), 1.0)
        nc.vector.tensor_copy(out=wc_sl(1), in_=kvc3)
        nc.vector.tensor_scalar(out=X2c3, in0=kvc3, scalar1=2.0, scalar2=None,
                                op0=OP.mult)
        nc.vector.tensor_tensor(out=wc_sl(2), in0=X2c3, in1=wc_sl(1), op=OP.mult)
        nc.vector.tensor_scalar(out=wc_sl(2), in0=wc_sl(2), scalar1=1.0,
                                scalar2=None, op0=OP.subtract)
        for r in range(3, RC):
            nc.vector.tensor_tensor(out=wc_sl(r), in0=X2c3, in1=wc_sl(r - 1),
                                    op=OP.mult)
            nc.vector.tensor_tensor(out=wc_sl(r), in0=wc_sl(r), in1=wc_sl(r - 2),
                                    op=OP.subtract)
        Vc = sb.tile([NCH, NKV], F16, tag="Vc")
        for t in range(NT):
            tp = psMix.tile([P, 512], F16, tag="mixT")
            nc.tensor.transpose(tp[0:NCH, 0:P], Wc[:, t * NCH:(t + 1) * NCH], ident)
            nc.scalar.copy(Vc[:, t * P:(t + 1) * P], tp[0:NCH, 0:P])

        # ------------------------------------------------ q-side features
        # q1x = q @ Wq1.T / sx  (n-major), then Chebyshev levels Qf [128, (l,c)]
        q1p = psMix.tile([P, 512], F32, tag="mix")
        nc.tensor.matmul(q1p[:, 0:C], qT, wq1T, start=True, stop=True)
        Qf = sb.tile([P, L * C], F16, tag="Qf")

        def qf_sl(l):
            return Qf[:, l * C:(l + 1) * C]

        X2q = sb.tile([P, C], F16, tag="X2q")
        nc.vector.memset(qf_sl(0), 1.0)
        nc.vector.tensor_copy(out=qf_sl(1), in_=q1p[:, 0:C])
        nc.vector.tensor_scalar(out=X2q, in0=qf_sl(1), scalar1=2.0, scalar2=None,
                                op0=OP.mult)
        nc.vector.tensor_tensor(out=qf_sl(2), in0=X2q, in1=qf_sl(1), op=OP.mult)
        nc.vector.tensor_scalar(out=qf_sl(2), in0=qf_sl(2), scalar1=1.0,
                                scalar2=None, op0=OP.subtract)
        for l in range(3, L):
            nc.vector.tensor_tensor(out=qf_sl(l), in0=X2q, in1=qf_sl(l - 1),
                                    op=OP.mult)
            nc.vector.tensor_tensor(out=qf_sl(l), in0=qf_sl(l), in1=qf_sl(l - 2),
                                    op=OP.subtract)

        # coord q-side features Qfc [128, (l,t)]
        Qfc = sb.tile([P, L * 3], F16, tag="Qfc")

        def qfc_sl(l):
            return Qfc[:, l * 3:(l + 1) * 3]

        X2qc = sb.tile([P, 3], F16, tag="X2qc")
        nc.vector.memset(qfc_sl(0), 1.0)
        nc.vector.tensor_copy(out=qfc_sl(1), in_=qc)
        nc.vector.tensor_scalar(out=X2qc, in0=qc, scalar1=2.0, scalar2=None,
                                op0=OP.mult)
        nc.vector.tensor_tensor(out=qfc_sl(2), in0=X2qc, in1=qfc_sl(1), op=OP.mult)
        nc.vector.tensor_scalar(out=qfc_sl(2), in0=qfc_sl(2), scalar1=1.0,
                                scalar2=None, op0=OP.subtract)
        for l in range(3, L):
            nc.vector.tensor_tensor(out=qfc_sl(l), in0=X2qc, in1=qfc_sl(l - 1),
                                    op=OP.mult)
            nc.vector.tensor_tensor(out=qfc_sl(l), in0=qfc_sl(l), in1=qfc_sl(l - 2),
                                    op=OP.subtract)

        # transpose Qf -> QfT chunks, fold with M1 -> GT chunks (iter-1 lhsT)
        QfT = []
        for i in range(NI):
            tp = psMix.tile([P, 512], F16, tag="mixT")
            nc.tensor.transpose(tp[:, 0:P], Qf[:, i * P:(i + 1) * P], ident)
            t16 = sb.tile([P, P], F16, tag=f"QfT{i}")
            nc.scalar.copy(t16, tp[:, 0:P])
            QfT.append(t16)
        GT = []
        for j in range(NJ):
            gp = psMix.tile([P, 512], F32, tag="mix")
            for i in range(NI):
                m1b = wpk[:, _OFF_M1 + (j * NI + i) * P:_OFF_M1 + (j * NI + i + 1) * P]
                nc.tensor.matmul(gp[:, 0:P], m1b, QfT[i],
                                 start=(i == 0), stop=(i == NI - 1))
            g16 = sb.tile([P, P], F16, tag=f"GT{j}")
            nc.vector.tensor_copy(out=g16, in_=gp[:, 0:P])
            GT.append(g16)
        # coord fold (both iterations; wds folded into mc1/mc2 on host)
        tpc = psMix.tile([P, 512], F16, tag="mixT")
        nc.tensor.transpose(tpc[0:L * 3, 0:P], Qfc, ident)
        QfcT = sb.tile([L * 3, P], F16, tag="QfcT")
        nc.scalar.copy(QfcT, tpc[0:L * 3, 0:P])
        GTc = []
        for it in range(ITERS):
            gp = psMix.tile([P, 512], F32, tag="mix")
            nc.tensor.matmul(gp[0:NCH, 0:P], (mc1 if it == 0 else mc2), QfcT,
                             start=True, stop=True)
            g16 = sb.tile([NCH, P], F16, tag=f"GTc{it}")
            nc.vector.tensor_copy(out=g16, in_=gp[0:NCH, 0:P])
            GTc.append(g16)

        # ------------------------------------------------ k-side iter-1 V
        # k1T = (Wkv1[:C]/sy) @ kv^T  -> x = chunk0 rows 32:64
        k1p = psBig.tile([P, NKV], F32, tag="big")
        for h in range(4):
            nc.tensor.matmul(k1p[0:C, h * 512:(h + 1) * 512], wk1T,
                             kvT[:, h * 512:(h + 1) * 512], start=True, stop=True)
        V0 = sb.tile([P, NKV], F16, tag="V0")
        V1 = sb.tile([P, NKV], F16, tag="V1")
        V2t = sb.tile([P, NKV], F16, tag="V2t")
        Vch = [V0, V1, V2t]
        # seed planes in separate base-partition-0 tiles (tensor_tensor
        # requires both SBUF inputs at the same base partition), then copy
        # into the chunk layouts.
        T4n = sb.tile([C, NKV], F16, tag="T4n")
        X2 = sb.tile([C, NKV], F16, tag="X2")
        xt = sb.tile([C, NKV], F16, tag="xt")
        t2 = sb.tile([C, NKV], F16, tag="t2")
        t3 = sb.tile([C, NKV], F16, tag="t3")
        Cm1 = sb.tile([P, NKV], F16, tag="Cm1")
        nc.scalar.copy(xt, k1p[0:C, :])          # f32 PSUM -> f16, ScalarE
        nc.vector.memset(V0[0:C, :], 1.0)        # T0
        nc.vector.tensor_scalar(out=X2, in0=xt, scalar1=2.0, scalar2=None,
                                op0=OP.mult)
        nc.vector.tensor_tensor(out=t2, in0=X2, in1=xt, op=OP.mult)
        nc.vector.tensor_scalar(out=t2, in0=t2, scalar1=1.0, scalar2=None,
                                op0=OP.subtract)
        nc.vector.tensor_tensor(out=t3, in0=X2, in1=t2, op=OP.mult)
        nc.vector.tensor_tensor(out=t3, in0=t3, in1=xt, op=OP.subtract)
        nc.vector.tensor_tensor(out=T4n, in0=X2, in1=t3, op=OP.mult)
        nc.vector.tensor_tensor(out=T4n, in0=T4n, in1=t2, op=OP.subtract)
        nc.vector.tensor_copy(out=V0[C:2 * C, :], in_=xt)
        nc.vector.tensor_copy(out=V0[2 * C:3 * C, :], in_=t2)
        nc.vector.tensor_copy(out=V0[3 * C:4 * C, :], in_=t3)
        # chunk_{-1} = [T4, T3, T2, T1] for the first stride-4 step
        nc.vector.tensor_copy(out=Cm1[0:C, :], in_=T4n)
        nc.vector.tensor_copy(out=Cm1[C:2 * C, :], in_=t3)
        nc.vector.tensor_copy(out=Cm1[2 * C:3 * C, :], in_=t2)
        nc.vector.tensor_copy(out=Cm1[3 * C:4 * C, :], in_=xt)
        # T4X2 = 2*T4 replicated to 128 partitions via repM matmul
        T4X2 = sb.tile([P, NKV], F16, tag="T4X2")
        for qtr in range(4):
            rp = psMix.tile([P, 512], F32, tag="mix")
            nc.tensor.matmul(rp, repM, T4n[:, qtr * 512:(qtr + 1) * 512],
                             start=True, stop=True)
            nc.scalar.copy(T4X2[:, qtr * 512:(qtr + 1) * 512], rp)
        prev2, prev1 = Cm1, V0
        for j in range(1, NJ):
            cur = Vch[j]
            nc.vector.tensor_tensor(out=cur, in0=T4X2, in1=prev1, op=OP.mult)
            nc.vector.tensor_tensor(out=cur, in0=cur, in1=prev2, op=OP.subtract)
            prev2, prev1 = prev1, cur

        # ------------------------------------------------ iter-2 V (linearized)
        # f = e^{-|k2|}, g = sign(k2) f  (ScalarE-heavy, runs alongside V build)
        k2p = psBig.tile([P, NKV], F32, tag="big")
        for h in range(4):
            nc.tensor.matmul(k2p[0:C, h * 512:(h + 1) * 512], wk2T,
                             kvT[:, h * 512:(h + 1) * 512], start=True, stop=True)
        Vl2 = sb.tile([2 * C, NKV], F16, tag="Vl2")
        a2 = sb.tile([C, NKV], F16, tag="a2")
        s2 = sb.tile([C, NKV], F16, tag="s2")
        nc.scalar.activation(a2, k2p[0:C, :], AF.Abs)
        nc.scalar.activation(s2, k2p[0:C, :], AF.Sign)
        nc.scalar.activation(Vl2[0:C, :], a2, AF.Exp, scale=-1.0)
        nc.vector.tensor_tensor(out=Vl2[C:2 * C, :], in0=s2, in1=Vl2[0:C, :],
                                op=OP.mult)

        # v matrices [128, (t, 33)] = [v | ones], m-major
        vmats = []
        for it, wv in ((0, wv1T), (1, wv2T)):
            vm = sb.tile([P, NT * (C + 1)], F16, tag=f"vm{it}")
            nc.vector.memset(vm, 1.0)
            vp = psMix.tile([P, 512], F32, tag="mix")
            for t in range(NT):
                nc.tensor.matmul(vp[:, t * C:(t + 1) * C], kvT[:, t * P:(t + 1) * P],
                                 wv, start=True, stop=True)
            vm3 = vm[:, :].rearrange("p (t c) -> p t c", t=NT, c=C + 1)[:, :, 0:C]
            vp3 = vp[:, :].rearrange("p (t c) -> p t c", t=NT, c=C)
            nc.vector.tensor_copy(out=vm3, in_=vp3)
            vmats.append(vm)

        # ------------------------------------------------ iterations
        G2 = sb.tile([2 * C, P], F16, tag="G2")   # iter-2 lhsT [ones; q2^T]
        nc.vector.memset(G2[0:C, :], 1.0)

        for it in range(ITERS):
            S = psBig.tile([P, NKV], F32, tag="big")
            if it == 0:
                chunks = [(GTc[0], Vc)] + [(GT[j], Vch[j]) for j in range(NJ)]
            else:
                chunks = [(GTc[1], Vc), (G2, Vl2)]
            nch = len(chunks)
            for ci, (lhsT, rhs) in enumerate(chunks):
                for h in range(4):
                    nc.tensor.matmul(S[:, h * 512:(h + 1) * 512], lhsT,
                                     rhs[:, h * 512:(h + 1) * 512],
                                     start=(ci == 0), stop=(ci == nch - 1))
            Pm = sb2.tile([P, NKV], F16, tag="Pm")
            nc.scalar.activation(Pm, S, AF.Exp, scale=SCALE)
            # P^T tile-wise + attn @ [v|ones] accumulation
            oTt = psMix.tile([P, 512], F32, tag="mix")
            oT = oTt[0:C + 1, 0:P]
            vm = vmats[it]
            for t in range(NT):
                tp = psMix.tile([P, 512], F16, tag="mixT")
                nc.tensor.transpose(tp[:, 0:P], Pm[:, t * P:(t + 1) * P], ident)
                pt16 = sb2.tile([P, P], F16, tag="pt16")
                nc.vector.tensor_copy(out=pt16, in_=tp[:, 0:P])
                nc.tensor.matmul(oT, vm[:, t * (C + 1):(t + 1) * (C + 1)], pt16,
                                 start=(t == 0), stop=(t == NT - 1))
            # normalize: o^T[0:32] * (1/denom) broadcast via rank-1 matmul
            rec = sb2.tile([1, P], F32, tag="rec")
            nc.vector.reciprocal(rec, oT[C:C + 1, :])
            ones1 = sb2.tile([1, C], F32, tag="ones1")
            nc.vector.memset(ones1, 1.0)
            rp = psMix.tile([P, 512], F32, tag="mix")
            nc.tensor.matmul(rp[0:C, 0:P], ones1, rec, start=True, stop=True)
            Rm = sb2.tile([C, P], F32, tag="Rm")
            nc.vector.tensor_copy(out=Rm, in_=rp[0:C, 0:P])
            oTn = sb2.tile([C, P], F16, tag="oTn")
            nc.vector.tensor_tensor(out=oTn, in0=oT[0:C, :], in1=Rm, op=OP.mult)
            if it == 0:
                # q2^T = Wq2 @ oTn  (stays transposed for iter-2 lhsT)
                qp2 = psMix.tile([P, 512], F32, tag="mix")
                nc.tensor.matmul(qp2[0:C, 0:P], wq2T, oTn, start=True, stop=True)
                nc.vector.tensor_copy(out=G2[C:2 * C, :], in_=qp2[0:C, 0:P])
            else:
                yp = psMix.tile([P, 512], F32, tag="mix")
                nc.tensor.matmul(yp[:, 0:C], oTn, wpT, start=True, stop=True)
                y_sb = sb2.tile([P, C], F32, tag="y_sb")
                nc.vector.tensor_tensor(out=y_sb, in0=yp[:, 0:C], in1=bpb,
                                        op=OP.add)
                nc.sync.dma_start(out=y_d.ap(), in_=y_sb)

        if reps > 1:
            _loop.__exit__(None, None, None)

    nc.compile()
    return nc


def make_in_maps(q, q_coord, kv, kv_coord, Wq, Wkv, Wdelta, Wp, bp):
    """Host-side sharding/layout prep. Core r handles batch r//4, rows (r%4)*128:."""
    q = np.asarray(q, np.float32)
    q_coord = np.asarray(q_coord, np.float32)
    kv = np.asarray(kv, np.float32)
    kv_coord = np.asarray(kv_coord, np.float32)
    Wq = np.asarray(Wq, np.float32)
    Wkv = np.asarray(Wkv, np.float32)
    Wdelta = np.asarray(Wdelta, np.float32)
    Wp = np.asarray(Wp, np.float32)
    bp = np.asarray(bp, np.float32)
    wds = Wdelta.sum(axis=1)  # [ITERS, 3]

    wpack = np.zeros((P, WPACK_COLS), np.float16)
    wpack[0:C, _OFF_WQ1:_OFF_WQ1 + C] = (Wq[0].T / SX1).astype(np.float16)
    wpack[0:C, _OFF_WQ2:_OFF_WQ2 + C] = Wq[1].T.astype(np.float16)
    wpack[0:ICO, _OFF_WK1:_OFF_WK1 + C] = (Wkv[0][:C].T / SY).astype(np.float16)
    wpack[0:ICO, _OFF_WV1:_OFF_WV1 + C] = Wkv[0][C:].T.astype(np.float16)
    wpack[0:ICO, _OFF_WK2:_OFF_WK2 + C] = Wkv[1][:C].T.astype(np.float16)
    wpack[0:ICO, _OFF_WV2:_OFF_WV2 + C] = Wkv[1][C:].T.astype(np.float16)
    wpack[0:C, _OFF_WP:_OFF_WP + C] = Wp.T.astype(np.float16)
    # replicate matrix: rep[c, p] = 2 * (p % 32 == c)
    rep = np.zeros((C, P), np.float16)
    for g in range(4):
        rep[:, g * C:(g + 1) * C] = 2.0 * np.eye(C, dtype=np.float16)
    wpack[0:C, _OFF_REP:_OFF_REP + P] = rep
    # coord fold matrices Mc_i[(l,t), (r,t)] = Bc[l,r] * wds[i,t]
    for i, off in ((0, _OFF_MC1), (1, _OFF_MC2)):
        mc = np.zeros((3 * L, NCH), np.float32)
        for l in range(L):
            for r in range(RC):
                for t in range(3):
                    mc[l * 3 + t, r * 3 + t] = _BC[l, r] * wds[i, t]
        wpack[0:3 * L, off:off + NCH] = mc.astype(np.float16)
    # main fold blocks M1b(i,j)[(lm,c), (rm,c')] = B1[4i+lm, 4j+rm] delta_cc'
    eye = np.eye(C, dtype=np.float32)
    for j in range(R // 4):
        for i in range(L // 4):
            blk = np.zeros((P, P), np.float32)
            for lm in range(4):
                for rm in range(4):
                    blk[lm * C:(lm + 1) * C, rm * C:(rm + 1) * C] = \
                        _B1[4 * i + lm, 4 * j + rm] * eye
            o = _OFF_M1 + (j * (L // 4) + i) * P
            wpack[:, o:o + P] = blk.astype(np.float16)

    bpb = np.broadcast_to(bp, (P, C)).astype(np.float32).copy()

    in_maps = []
    for rcore in range(NCORES):
        b, jj = divmod(rcore, NQ // P)
        rows = slice(jj * P, (jj + 1) * P)
        kvc_m = np.zeros((P, NT * 3), np.float16)
        kvcb = (kv_coord[b] / SYC).reshape(NT, P, 3)
        for t in range(NT):
            kvc_m[:, t * 3:(t + 1) * 3] = kvcb[t].astype(np.float16)
        in_maps.append({
            "qT16": q[b, rows].T.astype(np.float16).copy(),
            "qc_n": (q_coord[b, rows] / SXC).astype(np.float16).copy(),
            "kvT16": kv[b].T.astype(np.float16).copy(),
            "kvc_m": kvc_m,
            "wpack": wpack,
            "bpb": bpb,
        })
    return in_maps


_PROGRAM = None


def kernel(q, q_coord, kv, kv_coord, Wq, Wkv, Wdelta, Wp, bp):
    global _PROGRAM
    if _PROGRAM is None:
        _PROGRAM = build_program()
    in_maps = make_in_maps(q, q_coord, kv, kv_coord, Wq, Wkv, Wdelta, Wp, bp)
    res = run_bass_kernel_spmd(_PROGRAM, in_maps, core_ids=list(range(NCORES)))
    out = np.empty((B, NQ, C), np.float32)
    for r in range(NCORES):
        b, j = divmod(r, NQ // P)
        out[b, j * P:(j + 1) * P, :] = res.results[r]["y"]
    return out
